# revision 35
# baseline (speedup 1.0000x reference)
"""Distributed causal multi-head attention for 8 Trainium2 NeuronCores.

Problem: B=2, S=2048, D=1024, H=16 heads (hd=64), fp32.
    qkv = x @ w_qkv + b_qkv ; causal softmax attention ; out = attn @ w_proj + b_proj

Distribution: core c -> (batch b = c//4, head group g = c%4 -> heads [4g, 4g+4)).
Each core runs QKV projection + attention for its 4 heads over the full
sequence of its batch, all in a transposed dataflow (channels on partitions,
sequence on the free axis) so no on-device transposes are needed (x arrives
host-transposed).  Key tricks:
  - Scores are computed as scores^T [kv, q] so the exp'd probabilities feed
    the PV matmul directly (no on-chip transposes anywhere).
  - No max-subtraction in softmax (logits are O(1) for this problem) and the
    softmax denominators come free as a 65th "ones" column appended to V.
  - bf16 matmul operands everywhere with f32 PSUM accumulation; the two heads
    of a pair run concurrently in the PE array via 64-row tile_position
    packing (contraction dim is hd=64).
  - Causal structure is identical on every core (SPMD-safe): q-tiles are
    grouped {k, k+4, k+8, k+12} via strided APs, kv-tiles beyond each q-tile's
    diagonal are skipped by suffix-sliced matmuls, and a single 128x128
    additive mask handles the diagonal tiles.
  - Each core outputs rows [256c, 256c+256) of BOTH batches, which turns the
    channel exchange into one full 8-rank AllToAll with zero padding; it is
    split in two halves (even/odd q-tiles, attention k-order 0,2,1,3) so the
    first A2A and the first half of the output projection overlap the second
    half of attention.  Outputs are disjoint row-slices; host concatenates.
"""

import os
import sys

sys.path.insert(0, "/opt/trn_rl_repo")

import numpy as np

import concourse.bass as bass
import concourse.tile as tile
from concourse import bacc, mybir
from concourse.bass_utils import run_bass_kernel_spmd

B, S, D = 2, 2048, 1024
H = 16
HD = 64
P = 128
N_CORES = 8
HPC = 4           # heads per core
SC = S // 4       # output rows per core (512)
DCH = D // P      # 8 contraction chunks
NQT = S // P      # 16 q tiles of 128
SCALE = 1.0 / 8.0  # 1/sqrt(hd)
NEG = -1.0e9

F32 = mybir.dt.float32
F32R = mybir.dt.float32r
BF16 = mybir.dt.bfloat16


def _patch_ldw_opt():
    """Enable walrus's LDWEIGHTS optimization (concourse hardcodes it off)."""
    if os.environ.get("KERNEL_LDW_OPT", "0") != "1":
        return
    import concourse.bass_utils as bu

    if getattr(bu, "_ldw_patched", False):
        return
    orig = bu.run_command

    def patched(cmd, *a, **k):
        cmd = [
            c.replace("--enable-ldw-opt=false", "--enable-ldw-opt=true")
            if isinstance(c, str)
            else c
            for c in cmd
        ]
        return orig(cmd, *a, **k)

    bu.run_command = patched
    bu._ldw_patched = True


def build():
    _patch_ldw_opt()
    nc = bacc.Bacc(num_devices=N_CORES)

    xT = nc.declare_dram_parameter("xT", [D, S], BF16, isOutput=False)
    w_qk = nc.declare_dram_parameter("w_qk", [D, 2 * HPC * HD], BF16, isOutput=False)
    w_v = nc.declare_dram_parameter("w_v", [D, HPC * HD], BF16, isOutput=False)
    consts = nc.declare_dram_parameter("consts", [P, 132], F32, isOutput=False)
    b_v = nc.declare_dram_parameter("b_v", [1, HPC * HD], BF16, isOutput=False)
    w_proj = nc.declare_dram_parameter("w_proj", [D, D], BF16, isOutput=False)
    b_proj = nc.declare_dram_parameter("b_proj", [1, D], BF16, isOutput=False)
    # each core outputs rows [256c, 256c+256) of BOTH batches
    out_ext = nc.declare_dram_parameter("out", [2, S // 8, D], F32, isOutput=True)

    groups = [list(range(N_CORES))]

    with tile.TileContext(nc) as tc:
        with (
            tc.tile_pool(name="weights", bufs=1) as wpool,
            tc.tile_pool(name="xslab", bufs=4) as xpool,
            tc.tile_pool(name="qkT", bufs=1) as qkpool,
            tc.tile_pool(name="big", bufs=1) as bigpool,
            tc.tile_pool(name="prob", bufs=5) as ppool,
            tc.tile_pool(name="small", bufs=4) as spool,
            tc.tile_pool(name="dram", bufs=1, space="DRAM") as dpool,
            tc.tile_pool(name="psA", bufs=3, space="PSUM") as psA,     # scores pairs, 2 banks/slot
            tc.tile_pool(name="psB", bufs=2, space="PSUM") as psB,     # accumulators/pv 1 bank
        ):
            SB = S // 8  # 256-row output slice per core (per batch)
            a2a_in = dpool.tile([N_CORES, HPC * HD, P], BF16, tag="a2a_in")
            a2a_out = dpool.tile([N_CORES, HPC * HD, P], BF16, tag="a2a_out")
            # ---- first x slab, then weights (DMA queue order = time order) ----
            xsl_list = [
                xpool.tile([P, DCH, 512], BF16, tag="xslab", name=f"xsl{st}")
                for st in range(4)
            ]
            nc.sync.dma_start(
                out=xsl_list[0][:],
                in_=xT[:, :].rearrange("(o p) s -> p o s", p=P)[:, :, 0:512],
            )
            wqk_sb = wpool.tile([P, DCH, 2 * HPC * HD], BF16)
            nc.sync.dma_start(out=wqk_sb[:], in_=w_qk[:, :].rearrange("(o p) c -> p o c", p=P))
            wv_sb = wpool.tile([P, DCH, HPC * HD], BF16)
            nc.sync.dma_start(out=wv_sb[:], in_=w_v[:, :].rearrange("(o p) c -> p o c", p=P))
            bv_sb = wpool.tile([1, HPC * HD], BF16)
            nc.sync.dma_start(out=bv_sb[:], in_=b_v[:, :])
            consts_sb = wpool.tile([P, 132], F32)
            nc.sync.dma_start(out=consts_sb[:], in_=consts[:, :])
            bqk_sb = consts_sb[:, 0:4]
            mask_sb = wpool.tile([P, P], F32)
            nc.vector.tensor_copy(out=mask_sb[:], in_=consts_sb[:, 4:132])
            ones_sb = wpool.tile([1, P], BF16)
            nc.vector.memset(ones_sb[:], 1.0)

            # qkT layout: [128, 4 coltiles, 2048 s]; coltiles 0-1 = q (256 ch), 2-3 = k
            qkT_sb = qkpool.tile([P, 4, S], BF16)
            # V': [128 kv_inner, 16 kv_outer, 4*65] bf16; per head h cols [65h,65h+64)=V, col 65h+64 = 1.0
            v1_sb = bigpool.tile([P, NQT, HPC * 65], BF16)
            nc.gpsimd.memset(v1_sb[:], 1.0)

            # ---- phase 1+2: QKV projections (single pass over xT slabs) ----
            for st in range(4):  # s in chunks of 512
                xsl = xsl_list[st]
                if st > 0:
                    nc.sync.dma_start(
                        out=xsl[:],
                        in_=xT[:, :].rearrange("(o p) s -> p o s", p=P)[:, :, st * 512:(st + 1) * 512],
                    )
                # qkT: out[col, s] accumulated over D chunks; w stationary
                for ct in range(4):
                    ps = psB.tile([P, 512], F32, tag="mm")
                    for d in range(DCH):
                        nc.tensor.matmul(
                            ps[:],
                            wqk_sb[:, d, ct * P:(ct + 1) * P],
                            xsl[:, d, :],
                            start=(d == 0),
                            stop=(d == DCH - 1),
                        )
                    nc.vector.tensor_scalar_add(
                        qkT_sb[:, ct, st * 512:(st + 1) * 512], ps[:], bqk_sb[:, ct:ct + 1]
                    )
                # V natural: out[s, vcol]; xT stationary
                for sq in range(4):  # s in chunks of 128 within this slab
                    t16 = st * 4 + sq
                    ps_full = psB.tile([P, 512], F32, tag="mm", name="vacc")
                    ps = ps_full[:, :HPC * HD]
                    nc.tensor.matmul(  # open with the b_v broadcast (K=1)
                        ps[:], ones_sb[:, :], bv_sb[:, :], start=True, stop=False
                    )
                    for d in range(DCH):
                        nc.tensor.matmul(
                            ps[:],
                            xsl[:, d, sq * P:(sq + 1) * P],
                            wv_sb[:, d, :],
                            start=False,
                            stop=(d == DCH - 1),
                        )
                    for h in range(HPC):
                        nc.vector.tensor_copy(
                            out=v1_sb[:, t16, h * 65:h * 65 + HD],
                            in_=ps[:, h * HD:(h + 1) * HD],
                        )

            # ---- phase 3: attention, transposed flash-style ----
            # attn_outT: [128, 2 ctile, 2048 s] bf16 (4 heads = 256 channels)
            aT_sb = bigpool.tile([P, 2, S], BF16)

            def attn_group(pair, k):
                """scores+softmax+PV for heads (2*pair, 2*pair+1), q-tile group k"""
                T = 13 + k
                qvA = qkT_sb[0:HD, pair, :].rearrange("p (i g) -> p i g", g=512)
                qvB = qkT_sb[HD:P, pair, :].rearrange("p (i g) -> p i g", g=512)
                kv_ct = 2 + pair
                pvA = psB.tile([P, 512], F32, tag="mm")
                pvB = psB.tile([P, 512], F32, tag="mm")
                for t in range(T):
                    s0 = max(0, (t - k + 3) // 4)
                    N = (4 - s0) * P
                    sc_full = psA.tile([P, 2, 512], F32, tag="sc")
                    sc = sc_full[:, :, :N]
                    nc.tensor.matmul(
                        sc[:, 0, :],
                        qkT_sb[0:HD, kv_ct, t * P:(t + 1) * P],
                        qvA[:, s0:4, k * P:(k + 1) * P],
                        start=True, stop=True, tile_position=(0, 0),
                    )
                    nc.tensor.matmul(
                        sc[:, 1, :],
                        qkT_sb[HD:P, kv_ct, t * P:(t + 1) * P],
                        qvB[:, s0:4, k * P:(k + 1) * P],
                        start=True, stop=True, tile_position=(64, 0),
                    )
                    if t >= k and (t - k) % 4 == 0:
                        nc.vector.tensor_add(
                            out=sc[:, :, 0:P], in0=sc[:, :, 0:P],
                            in1=mask_sb[:, None, :].to_broadcast((P, 2, P)),
                        )
                    pr = ppool.tile([P, 2, N], BF16, tag="prob")
                    nc.scalar.activation(
                        pr[:], sc[:], mybir.ActivationFunctionType.Exp, scale=SCALE
                    )
                    for hh, pv in ((0, pvA), (1, pvB)):
                        h = 2 * pair + hh
                        nc.tensor.matmul(
                            pv[0:65, s0 * P:512],
                            v1_sb[:, t, h * 65:(h + 1) * 65],
                            pr[:, hh, :],
                            start=(t == 0), stop=(t == T - 1),
                        )
                for hh, pv in ((0, pvA), (1, pvB)):
                    h = 2 * pair + hh
                    base = (h % 2) * HD
                    # evacuate PSUM immediately so the pv slot frees for the
                    # next group before the normalize chain completes
                    sums_sb = spool.tile([1, 512], F32, tag="sums")
                    nc.vector.tensor_copy(out=sums_sb[:], in_=pv[64:65, :])
                    pvc = spool.tile([HD, 512], F32, tag="pvc")
                    nc.vector.tensor_copy(out=pvc[:], in_=pv[0:HD, :])
                    rec = spool.tile([1, 512], F32, tag="rec")
                    nc.vector.reciprocal_approx_fast(rec[:], sums_sb[:])
                    bc = spool.tile([HD, 512], F32, tag="bc")
                    nc.gpsimd.partition_broadcast(bc[:], rec[:])
                    nc.vector.tensor_tensor(
                        out=aT_sb[base:base + HD, h // 2, :]
                        .rearrange("p (i g) -> p i g", g=256)
                        [:, 4 * (k % 2):4 * (k % 2) + 4,
                         (k // 2) * P:(k // 2) * P + P],
                        in0=pvc[:].rearrange("p (i f) -> p i f", f=P),
                        in1=bc[:].rearrange("p (i f) -> p i f", f=P),
                        op=mybir.AluOpType.mult,
                    )

            def stage(a2a_buf, half):
                # shard p = my 256 channels x q-tile (2p + half); each parity
                # half is one contiguous block of aT -> one DMA per chan-tile
                for t0 in range(2):
                    nc.sync.dma_start(
                        out=a2a_buf[:, t0 * P:(t0 + 1) * P, :]
                        .rearrange("s pp f -> pp s f"),
                        in_=aT_sb[:, t0, half * 1024:(half + 1) * 1024]
                        .rearrange("pp (s f) -> pp s f", f=P),
                    )

            a2a_in2 = dpool.tile([N_CORES, HPC * HD, P], BF16, tag="a2a_in2")
            a2a_out2 = dpool.tile([N_CORES, HPC * HD, P], BF16, tag="a2a_out2")

            for k in (0, 2):
                for pair in range(2):
                    attn_group(pair, k)
            stage(a2a_in, 0)
            nc.gpsimd.collective_compute(
                "AllToAll", mybir.AluOpType.bypass,
                ins=[a2a_in[:].opt()], outs=[a2a_out[:].opt()],
                replica_groups=groups,
            )
            for k in (1, 3):
                for pair in range(2):
                    attn_group(pair, k)
            stage(a2a_in2, 1)
            nc.gpsimd.collective_compute(
                "AllToAll", mybir.AluOpType.bypass,
                ins=[a2a_in2[:].opt()], outs=[a2a_out2[:].opt()],
                replica_groups=groups,
            )

            # ---- phase 4: output projection (rows = q-tiles 2c (half0), 2c+1 (half1)) ----
            wproj_sb = wpool.tile([P, DCH, D], BF16)
            nc.sync.dma_start(out=wproj_sb[:], in_=w_proj[:, :].rearrange("(o p) c -> p o c", p=P))
            bproj_sb = wpool.tile([1, D], BF16)
            nc.sync.dma_start(out=bproj_sb[:], in_=b_proj[:, :])
            out_sb = bigpool.tile([P, 2, 2, D], F32)
            for sq, a2a_o in ((0, a2a_out), (1, a2a_out2)):
                for b2 in range(2):
                    pjT_sb = bigpool.tile([P, DCH, P], BF16, tag="pjT")
                    nc.sync.dma_start(
                        out=pjT_sb[:],
                        in_=a2a_o[b2 * 4:(b2 + 1) * 4, :, :]
                        .rearrange("g (t pp) f -> pp (g t) f", pp=P),
                    )
                    for dc in range(2):
                        ps = psB.tile([P, 512], F32, tag="mm", name="pacc")
                        nc.tensor.matmul(
                            ps[:],
                            ones_sb[:, :],
                            bproj_sb[:, dc * 512:(dc + 1) * 512],
                            start=True, stop=False,
                        )
                        for ch in range(DCH):
                            nc.tensor.matmul(
                                ps[:],
                                pjT_sb[:, ch, :],
                                wproj_sb[:, ch, dc * 512:(dc + 1) * 512],
                                start=False,
                                stop=(ch == DCH - 1),
                            )
                        nc.vector.tensor_copy(
                            out=out_sb[:, b2, sq, dc * 512:(dc + 1) * 512], in_=ps[:]
                        )
                # ship this row-half as soon as its copies land
                nc.sync.dma_start(
                    out=out_ext[:, sq * P:(sq + 1) * P, :].rearrange("b pp d -> pp b d"),
                    in_=out_sb[:, :, sq, :],
                )

    nc.compile()
    return nc


def make_in_maps(x, w_qkv, b_qkv, w_proj, b_proj):
    import ml_dtypes

    bf16 = ml_dtypes.bfloat16
    x = np.asarray(x, dtype=np.float32)
    w_qkv = np.asarray(w_qkv, dtype=np.float32)
    b_qkv = np.asarray(b_qkv, dtype=np.float32)
    w_proj_bf = np.ascontiguousarray(np.asarray(w_proj, dtype=np.float32).astype(bf16))
    b_proj_bf = np.ascontiguousarray(
        np.asarray(b_proj, dtype=np.float32).astype(bf16).reshape(1, -1)
    )

    # causal mask tile: mask[kv_local, q_local] = 0 if q >= kv else NEG
    m = np.where(np.arange(P)[None, :] >= np.arange(P)[:, None], 0.0, NEG).astype(np.float32)

    in_maps = []
    for c in range(N_CORES):
        b, g = divmod(c, 4)
        hs = slice(g * HPC * HD, (g + 1) * HPC * HD)
        xT = np.ascontiguousarray(x[b].T.astype(bf16))           # [D, S]
        w_q = w_qkv[:, 0:D][:, hs]
        w_k = w_qkv[:, D:2 * D][:, hs]
        w_qk = np.ascontiguousarray(np.concatenate([w_q, w_k], axis=1).astype(bf16))
        w_v = np.ascontiguousarray(w_qkv[:, 2 * D:3 * D][:, hs].astype(bf16))
        bq = np.concatenate([b_qkv[0:D][hs], b_qkv[D:2 * D][hs]])        # [512]
        bqk = bq.reshape(4, P).T                                         # [128, 4] f32
        cst = np.ascontiguousarray(np.concatenate([bqk, m], axis=1))     # [128, 132]
        bv = np.ascontiguousarray(b_qkv[2 * D:3 * D][hs].reshape(1, -1).astype(bf16))
        in_maps.append(
            {
                "xT": xT,
                "w_qk": w_qk,
                "w_v": w_v,
                "consts": cst,
                "b_v": bv,
                "w_proj": w_proj_bf,
                "b_proj": b_proj_bf,
            }
        )
    return in_maps


_NC_CACHE = None


def _install_ntff_shim():
    """Provide the antenv.axon_hooks module bass_utils wants for trace=True.

    The agent image's antenv package lacks axon_hooks; register a stub module
    holding the ctypes-based NTFF profile hook from the axon boot code.
    """
    import sys as _sys
    import types

    if "antenv.axon_hooks" in _sys.modules:
        return
    try:
        from trn_agent_boot.trn_boot import _ntff_profile_via_ctypes

        hook = _ntff_profile_via_ctypes("/opt/axon/libaxon_pjrt.so")
    except Exception:
        hook = None
    mod = types.ModuleType("antenv.axon_hooks")
    mod._hook = hook
    mod.get_axon_ntff_profile_hook = lambda: mod._hook
    mod.set_axon_ntff_profile_hook = lambda h: setattr(mod, "_hook", h)
    _sys.modules["antenv.axon_hooks"] = mod


def kernel(x, w_qkv, b_qkv, w_proj, b_proj):
    global _NC_CACHE
    if _NC_CACHE is None:
        _NC_CACHE = build()
    nc = _NC_CACHE
    in_maps = make_in_maps(x, w_qkv, b_qkv, w_proj, b_proj)
    trace = bool(int(os.environ.get("KERNEL_TRACE", "0")))
    if trace:
        _install_ntff_shim()
    res = run_bass_kernel_spmd(
        nc,
        in_maps,
        core_ids=list(range(N_CORES)),
        trace=trace,
    )
    out = np.empty((B, S, D), dtype=np.float32)
    SB = S // 8
    for c in range(N_CORES):
        oc = res.results[c]["out"]  # [2, 256, 1024]
        out[0, c * SB:(c + 1) * SB, :] = oc[0]
        out[1, c * SB:(c + 1) * SB, :] = oc[1]
    kernel.last_results = res
    return out


# revision 36
# speedup vs baseline: 1.0342x; 1.0342x over previous
"""Distributed causal multi-head attention for 8 Trainium2 NeuronCores.

Problem: B=2, S=2048, D=1024, H=16 heads (hd=64), fp32.
    qkv = x @ w_qkv + b_qkv ; causal softmax attention ; out = attn @ w_proj + b_proj

Distribution: core c -> (batch b = c//4, head group g = c%4 -> heads [4g, 4g+4)).
Each core runs QKV projection + attention for its 4 heads over the full
sequence of its batch, all in a transposed dataflow (channels on partitions,
sequence on the free axis) so no on-device transposes are needed (x arrives
host-transposed).  Key tricks:
  - Scores are computed as scores^T [kv, q] so the exp'd probabilities feed
    the PV matmul directly (no on-chip transposes anywhere).
  - No max-subtraction in softmax (logits are O(1) for this problem) and the
    softmax denominators come free as a 65th "ones" column appended to V.
  - bf16 matmul operands everywhere with f32 PSUM accumulation; the two heads
    of a pair run concurrently in the PE array via 64-row tile_position
    packing (contraction dim is hd=64).
  - Causal structure is identical on every core (SPMD-safe): q-tiles are
    grouped {k, k+4, k+8, k+12} via strided APs, kv-tiles beyond each q-tile's
    diagonal are skipped by suffix-sliced matmuls, and a single 128x128
    additive mask handles the diagonal tiles.
  - Each core outputs rows [256c, 256c+256) of BOTH batches, which turns the
    channel exchange into one full 8-rank AllToAll with zero padding; it is
    split in two halves (even/odd q-tiles, attention k-order 0,2,1,3) so the
    first A2A and the first half of the output projection overlap the second
    half of attention.  Outputs are disjoint row-slices; host concatenates.
"""

import os
import sys

sys.path.insert(0, "/opt/trn_rl_repo")

import numpy as np

import concourse.bass as bass
import concourse.tile as tile
from concourse import bacc, mybir
from concourse.bass_utils import run_bass_kernel_spmd

B, S, D = 2, 2048, 1024
H = 16
HD = 64
P = 128
N_CORES = 8
HPC = 4           # heads per core
SC = S // 4       # output rows per core (512)
DCH = D // P      # 8 contraction chunks
NQT = S // P      # 16 q tiles of 128
SCALE = 1.0 / 8.0  # 1/sqrt(hd)
NEG = -1.0e9

F32 = mybir.dt.float32
F32R = mybir.dt.float32r
BF16 = mybir.dt.bfloat16


def _patch_ldw_opt():
    """Enable walrus's LDWEIGHTS optimization (concourse hardcodes it off)."""
    if os.environ.get("KERNEL_LDW_OPT", "0") != "1":
        return
    import concourse.bass_utils as bu

    if getattr(bu, "_ldw_patched", False):
        return
    orig = bu.run_command

    def patched(cmd, *a, **k):
        cmd = [
            c.replace("--enable-ldw-opt=false", "--enable-ldw-opt=true")
            if isinstance(c, str)
            else c
            for c in cmd
        ]
        return orig(cmd, *a, **k)

    bu.run_command = patched
    bu._ldw_patched = True


def build():
    _patch_ldw_opt()
    nc = bacc.Bacc(num_devices=N_CORES)

    xT = nc.declare_dram_parameter("xT", [D, S], BF16, isOutput=False)
    w_qk = nc.declare_dram_parameter("w_qk", [D, 2 * HPC * HD], BF16, isOutput=False)
    w_v = nc.declare_dram_parameter("w_v", [D, HPC * HD], BF16, isOutput=False)
    consts = nc.declare_dram_parameter("consts", [P, 132], F32, isOutput=False)
    b_v = nc.declare_dram_parameter("b_v", [1, HPC * HD], BF16, isOutput=False)
    w_proj = nc.declare_dram_parameter("w_proj", [D, D], BF16, isOutput=False)
    b_proj = nc.declare_dram_parameter("b_proj", [1, D], BF16, isOutput=False)
    # each core outputs rows [256c, 256c+256) of BOTH batches
    out_ext = nc.declare_dram_parameter("out", [2, S // 8, D], F32, isOutput=True)

    groups = [list(range(N_CORES))]

    with tile.TileContext(nc) as tc:
        with (
            tc.tile_pool(name="weights", bufs=1) as wpool,
            tc.tile_pool(name="xslab", bufs=4) as xpool,
            tc.tile_pool(name="qkT", bufs=1) as qkpool,
            tc.tile_pool(name="big", bufs=1) as bigpool,
            tc.tile_pool(name="prob", bufs=3) as ppool,
            tc.tile_pool(name="small", bufs=3) as spool,
            tc.tile_pool(name="dram", bufs=1, space="DRAM") as dpool,
            tc.tile_pool(name="psA", bufs=3, space="PSUM") as psA,     # scores pairs, 2 banks/slot
            tc.tile_pool(name="psB", bufs=2, space="PSUM") as psB,     # accumulators/pv 1 bank
        ):
            SB = S // 8  # 256-row output slice per core (per batch)
            a2a_in = dpool.tile([N_CORES, HPC * HD, P], BF16, tag="a2a_in")
            a2a_out = dpool.tile([N_CORES, HPC * HD, P], BF16, tag="a2a_out")
            # ---- first x slab, then weights (DMA queue order = time order) ----
            xsl_list = [
                xpool.tile([P, DCH, 512], BF16, tag="xslab", name=f"xsl{st}")
                for st in range(4)
            ]
            nc.sync.dma_start(
                out=xsl_list[0][:],
                in_=xT[:, :].rearrange("(o p) s -> p o s", p=P)[:, :, 0:512],
            )
            wqk_sb = wpool.tile([P, DCH, 2 * HPC * HD], BF16)
            nc.sync.dma_start(out=wqk_sb[:], in_=w_qk[:, :].rearrange("(o p) c -> p o c", p=P))
            wv_sb = wpool.tile([P, DCH, HPC * HD], BF16)
            nc.sync.dma_start(out=wv_sb[:], in_=w_v[:, :].rearrange("(o p) c -> p o c", p=P))
            bv_sb = wpool.tile([1, HPC * HD], BF16)
            nc.sync.dma_start(out=bv_sb[:], in_=b_v[:, :])
            consts_sb = wpool.tile([P, 132], F32)
            nc.sync.dma_start(out=consts_sb[:], in_=consts[:, :])
            bqk_sb = consts_sb[:, 0:4]
            mask_sb = wpool.tile([P, P], F32)
            nc.vector.tensor_copy(out=mask_sb[:], in_=consts_sb[:, 4:132])
            ones_sb = wpool.tile([1, P], BF16)
            nc.vector.memset(ones_sb[:], 1.0)

            # qkT layout: [128, 4 coltiles, 2048 s]; coltiles 0-1 = q (256 ch), 2-3 = k
            qkT_sb = qkpool.tile([P, 4, S], BF16)
            # V': [128 kv_inner, 16 kv_outer, 4*65] bf16; per head h cols [65h,65h+64)=V, col 65h+64 = 1.0
            v1_sb = bigpool.tile([P, NQT, HPC * 65], BF16)
            nc.gpsimd.memset(v1_sb[:], 1.0)

            # ---- phase 1+2: QKV projections (single pass over xT slabs) ----
            for st in range(4):  # s in chunks of 512
                xsl = xsl_list[st]
                if st > 0:
                    nc.sync.dma_start(
                        out=xsl[:],
                        in_=xT[:, :].rearrange("(o p) s -> p o s", p=P)[:, :, st * 512:(st + 1) * 512],
                    )
                # qkT: out[col, s] accumulated over D chunks; w stationary
                for ct in range(4):
                    ps = psB.tile([P, 512], F32, tag="mm")
                    for d in range(DCH):
                        nc.tensor.matmul(
                            ps[:],
                            wqk_sb[:, d, ct * P:(ct + 1) * P],
                            xsl[:, d, :],
                            start=(d == 0),
                            stop=(d == DCH - 1),
                        )
                    nc.vector.tensor_scalar_add(
                        qkT_sb[:, ct, st * 512:(st + 1) * 512], ps[:], bqk_sb[:, ct:ct + 1]
                    )
                # V natural: out[s, vcol]; xT stationary
                for sq in range(4):  # s in chunks of 128 within this slab
                    t16 = st * 4 + sq
                    ps_full = psB.tile([P, 512], F32, tag="mm", name="vacc")
                    ps = ps_full[:, :HPC * HD]
                    nc.tensor.matmul(  # open with the b_v broadcast (K=1)
                        ps[:], ones_sb[:, :], bv_sb[:, :], start=True, stop=False
                    )
                    for d in range(DCH):
                        nc.tensor.matmul(
                            ps[:],
                            xsl[:, d, sq * P:(sq + 1) * P],
                            wv_sb[:, d, :],
                            start=False,
                            stop=(d == DCH - 1),
                        )
                    for h in range(HPC):
                        nc.vector.tensor_copy(
                            out=v1_sb[:, t16, h * 65:h * 65 + HD],
                            in_=ps[:, h * HD:(h + 1) * HD],
                        )

            # ---- phase 3: attention, transposed flash-style ----
            # attn_outT: [128, 2 ctile, 2048 s] bf16 (4 heads = 256 channels)
            aT_sb = bigpool.tile([P, 2, S], BF16)

            def attn_group(pair, k):
                """scores+softmax+PV for heads (2*pair, 2*pair+1), q-tile group k"""
                T = 13 + k
                qvA = qkT_sb[0:HD, pair, :].rearrange("p (i g) -> p i g", g=512)
                qvB = qkT_sb[HD:P, pair, :].rearrange("p (i g) -> p i g", g=512)
                kv_ct = 2 + pair
                pvA = psB.tile([P, 512], F32, tag="mm")
                pvB = psB.tile([P, 512], F32, tag="mm")
                for t in range(T):
                    s0 = max(0, (t - k + 3) // 4)
                    N = (4 - s0) * P
                    sc_full = psA.tile([P, 2, 512], F32, tag="sc")
                    sc = sc_full[:, :, :N]
                    nc.tensor.matmul(
                        sc[:, 0, :],
                        qkT_sb[0:HD, kv_ct, t * P:(t + 1) * P],
                        qvA[:, s0:4, k * P:(k + 1) * P],
                        start=True, stop=True, tile_position=(0, 0),
                    )
                    nc.tensor.matmul(
                        sc[:, 1, :],
                        qkT_sb[HD:P, kv_ct, t * P:(t + 1) * P],
                        qvB[:, s0:4, k * P:(k + 1) * P],
                        start=True, stop=True, tile_position=(64, 0),
                    )
                    if t >= k and (t - k) % 4 == 0:
                        nc.vector.tensor_add(
                            out=sc[:, :, 0:P], in0=sc[:, :, 0:P],
                            in1=mask_sb[:, None, :].to_broadcast((P, 2, P)),
                        )
                    pr = ppool.tile([P, 2, N], BF16, tag="prob")
                    nc.scalar.activation(
                        pr[:], sc[:], mybir.ActivationFunctionType.Exp, scale=SCALE
                    )
                    for hh, pv in ((0, pvA), (1, pvB)):
                        h = 2 * pair + hh
                        nc.tensor.matmul(
                            pv[0:65, s0 * P:512],
                            v1_sb[:, t, h * 65:(h + 1) * 65],
                            pr[:, hh, :],
                            start=(t == 0), stop=(t == T - 1),
                        )
                for hh, pv in ((0, pvA), (1, pvB)):
                    h = 2 * pair + hh
                    base = (h % 2) * HD
                    # evacuate PSUM immediately so the pv slot frees for the
                    # next group before the normalize chain completes
                    sums_sb = spool.tile([1, 512], F32, tag="sums")
                    nc.vector.tensor_copy(out=sums_sb[:], in_=pv[64:65, :])
                    pvc = spool.tile([HD, 512], F32, tag="pvc")
                    nc.vector.tensor_copy(out=pvc[:], in_=pv[0:HD, :])
                    rec = spool.tile([1, 512], F32, tag="rec")
                    nc.vector.reciprocal_approx_fast(rec[:], sums_sb[:])
                    bc = spool.tile([HD, 512], F32, tag="bc")
                    nc.gpsimd.partition_broadcast(bc[:], rec[:])
                    nc.vector.tensor_tensor(
                        out=aT_sb[base:base + HD, h // 2, :]
                        .rearrange("p (i g) -> p i g", g=256)
                        [:, 4 * (k % 2):4 * (k % 2) + 4,
                         (k // 2) * P:(k // 2) * P + P],
                        in0=pvc[:].rearrange("p (i f) -> p i f", f=P),
                        in1=bc[:].rearrange("p (i f) -> p i f", f=P),
                        op=mybir.AluOpType.mult,
                    )

            def stage(a2a_buf, half):
                # shard p = my 256 channels x q-tile (2p + half); each parity
                # half is one contiguous block of aT -> one DMA per chan-tile
                for t0 in range(2):
                    nc.sync.dma_start(
                        out=a2a_buf[:, t0 * P:(t0 + 1) * P, :]
                        .rearrange("s pp f -> pp s f"),
                        in_=aT_sb[:, t0, half * 1024:(half + 1) * 1024]
                        .rearrange("pp (s f) -> pp s f", f=P),
                    )

            a2a_in2 = dpool.tile([N_CORES, HPC * HD, P], BF16, tag="a2a_in2")
            a2a_out2 = dpool.tile([N_CORES, HPC * HD, P], BF16, tag="a2a_out2")

            for k in (0, 2):
                for pair in range(2):
                    attn_group(pair, k)
            stage(a2a_in, 0)
            nc.gpsimd.collective_compute(
                "AllToAll", mybir.AluOpType.bypass,
                ins=[a2a_in[:].opt()], outs=[a2a_out[:].opt()],
                replica_groups=groups,
            )
            for k in (1, 3):
                for pair in range(2):
                    attn_group(pair, k)
            stage(a2a_in2, 1)
            nc.gpsimd.collective_compute(
                "AllToAll", mybir.AluOpType.bypass,
                ins=[a2a_in2[:].opt()], outs=[a2a_out2[:].opt()],
                replica_groups=groups,
            )

            # ---- phase 4: output projection (rows = q-tiles 2c (half0), 2c+1 (half1)) ----
            wproj_sb = wpool.tile([P, DCH, D], BF16)
            nc.sync.dma_start(out=wproj_sb[:], in_=w_proj[:, :].rearrange("(o p) c -> p o c", p=P))
            bproj_sb = wpool.tile([1, D], BF16)
            nc.sync.dma_start(out=bproj_sb[:], in_=b_proj[:, :])
            out_sb = bigpool.tile([P, 2, 2, D], F32)
            for sq, a2a_o in ((0, a2a_out), (1, a2a_out2)):
                for b2 in range(2):
                    pjT_sb = bigpool.tile([P, DCH, P], BF16, tag="pjT")
                    nc.sync.dma_start(
                        out=pjT_sb[:],
                        in_=a2a_o[b2 * 4:(b2 + 1) * 4, :, :]
                        .rearrange("g (t pp) f -> pp (g t) f", pp=P),
                    )
                    for dc in range(2):
                        ps = psB.tile([P, 512], F32, tag="mm", name="pacc")
                        nc.tensor.matmul(
                            ps[:],
                            ones_sb[:, :],
                            bproj_sb[:, dc * 512:(dc + 1) * 512],
                            start=True, stop=False,
                        )
                        for ch in range(DCH):
                            nc.tensor.matmul(
                                ps[:],
                                pjT_sb[:, ch, :],
                                wproj_sb[:, ch, dc * 512:(dc + 1) * 512],
                                start=False,
                                stop=(ch == DCH - 1),
                            )
                        nc.vector.tensor_copy(
                            out=out_sb[:, b2, sq, dc * 512:(dc + 1) * 512], in_=ps[:]
                        )
                # ship this row-half as soon as its copies land
                nc.sync.dma_start(
                    out=out_ext[:, sq * P:(sq + 1) * P, :].rearrange("b pp d -> pp b d"),
                    in_=out_sb[:, :, sq, :],
                )

    nc.compile()
    return nc


def make_in_maps(x, w_qkv, b_qkv, w_proj, b_proj):
    import ml_dtypes

    bf16 = ml_dtypes.bfloat16
    x = np.asarray(x, dtype=np.float32)
    w_qkv = np.asarray(w_qkv, dtype=np.float32)
    b_qkv = np.asarray(b_qkv, dtype=np.float32)
    w_proj_bf = np.ascontiguousarray(np.asarray(w_proj, dtype=np.float32).astype(bf16))
    b_proj_bf = np.ascontiguousarray(
        np.asarray(b_proj, dtype=np.float32).astype(bf16).reshape(1, -1)
    )

    # causal mask tile: mask[kv_local, q_local] = 0 if q >= kv else NEG
    m = np.where(np.arange(P)[None, :] >= np.arange(P)[:, None], 0.0, NEG).astype(np.float32)

    in_maps = []
    for c in range(N_CORES):
        b, g = divmod(c, 4)
        hs = slice(g * HPC * HD, (g + 1) * HPC * HD)
        xT = np.ascontiguousarray(x[b].T.astype(bf16))           # [D, S]
        w_q = w_qkv[:, 0:D][:, hs]
        w_k = w_qkv[:, D:2 * D][:, hs]
        w_qk = np.ascontiguousarray(np.concatenate([w_q, w_k], axis=1).astype(bf16))
        w_v = np.ascontiguousarray(w_qkv[:, 2 * D:3 * D][:, hs].astype(bf16))
        bq = np.concatenate([b_qkv[0:D][hs], b_qkv[D:2 * D][hs]])        # [512]
        bqk = bq.reshape(4, P).T                                         # [128, 4] f32
        cst = np.ascontiguousarray(np.concatenate([bqk, m], axis=1))     # [128, 132]
        bv = np.ascontiguousarray(b_qkv[2 * D:3 * D][hs].reshape(1, -1).astype(bf16))
        in_maps.append(
            {
                "xT": xT,
                "w_qk": w_qk,
                "w_v": w_v,
                "consts": cst,
                "b_v": bv,
                "w_proj": w_proj_bf,
                "b_proj": b_proj_bf,
            }
        )
    return in_maps


_NC_CACHE = None


def _install_ntff_shim():
    """Provide the antenv.axon_hooks module bass_utils wants for trace=True.

    The agent image's antenv package lacks axon_hooks; register a stub module
    holding the ctypes-based NTFF profile hook from the axon boot code.
    """
    import sys as _sys
    import types

    if "antenv.axon_hooks" in _sys.modules:
        return
    try:
        from trn_agent_boot.trn_boot import _ntff_profile_via_ctypes

        hook = _ntff_profile_via_ctypes("/opt/axon/libaxon_pjrt.so")
    except Exception:
        hook = None
    mod = types.ModuleType("antenv.axon_hooks")
    mod._hook = hook
    mod.get_axon_ntff_profile_hook = lambda: mod._hook
    mod.set_axon_ntff_profile_hook = lambda h: setattr(mod, "_hook", h)
    _sys.modules["antenv.axon_hooks"] = mod


def kernel(x, w_qkv, b_qkv, w_proj, b_proj):
    global _NC_CACHE
    if _NC_CACHE is None:
        _NC_CACHE = build()
    nc = _NC_CACHE
    in_maps = make_in_maps(x, w_qkv, b_qkv, w_proj, b_proj)
    trace = bool(int(os.environ.get("KERNEL_TRACE", "0")))
    if trace:
        _install_ntff_shim()
    res = run_bass_kernel_spmd(
        nc,
        in_maps,
        core_ids=list(range(N_CORES)),
        trace=trace,
    )
    out = np.empty((B, S, D), dtype=np.float32)
    SB = S // 8
    for c in range(N_CORES):
        oc = res.results[c]["out"]  # [2, 256, 1024]
        out[0, c * SB:(c + 1) * SB, :] = oc[0]
        out[1, c * SB:(c + 1) * SB, :] = oc[1]
    kernel.last_results = res
    return out
